# revision 69
# baseline (speedup 1.0000x reference)
"""TRN2 Bass kernel for nn_LogDomainResNet — fp16 main + fp8e5m2 DoubleRow corrections.

Real-domain math (slog plumbing cancels exactly):

    v0      = sign_x * exp(log_abs_x)
    v_{i+1} = tanh(v_i @ W_i + b_i) + v_i        (7 inner layers)
    t       = v_7 @ W_final
    out     = stack([sign(t), log|t|])

Precision scheme per matmul (t = x @ W), all accumulating into one fp32 PSUM
bank:
  - state kept as an exact fp16 pair: xh = fp16(x), xl = fp16(x - xh)
  - pass 1 (fp16, full rate): xh @ Wh, Wh = fp16(W) — zero operand rounding
  - pass 2 (fp8e5m2 DoubleRow, half rate): the two first-order corrections
        xl @ Wh   via  e5m2(xl*2^3) @ e5m2(Wh*2^-3)
        xh @ Wres via  e5m2(xh*2^-6) @ e5m2(Wres*2^6),  Wres = W - Wh
    scales multiply out to 1, so the corrections accumulate directly.
  Per-layer error ~2^-14.5; full-pipeline rel err ~9e-3 (sim).

Layout: activations transposed [feature -> partitions, batch -> free].
DoubleRow contracts pairs (partition p, slot i): state/weight rows use a
fixed permutation P (d = 256j + 2p + i  <->  m = 128*(2j+i) + p) applied
host-side to W rows AND columns (inner layers), so matmul outputs land
directly in state order and the skip-add never crosses partitions. The
final layer keeps natural columns, emitting t in natural [batch, feature].

Sharding: data-parallel over batch, 1024 rows per core x 8 cores.
"""

import numpy as np

_B, _D, _NL = 8192, 1024, 8
_NCORES = 8
_BP = _B // _NCORES
_P = 128
_KC = _D // _P               # 8 state chunks (blk)
_BT = _BP // _P              # 8 batch tiles
_BCH = 512                   # PSUM free dim
_NBC = _BP // _BCH           # 2 batch chunks per layer pass

_cached_nc = None
last_results = None


def _build():
    import concourse.mybir as mybir
    from concourse import bacc
    from concourse.tile import TileContext
    from concourse.masks import make_identity

    f32, f16, f8e5 = mybir.dt.float32, mybir.dt.float16, mybir.dt.float8e5
    AF = mybir.ActivationFunctionType
    DR = mybir.MatmulPerfMode.DoubleRow

    nc = bacc.Bacc("TRN2", target_bir_lowering=False, debug=False)
    d_sgn = nc.dram_tensor("sign_x", [_BP, _D], f8e5, kind="ExternalInput")
    d_lab = nc.dram_tensor("log_abs_x", [_BP, _D], f32, kind="ExternalInput")
    d_wh = nc.dram_tensor("wh", [_NL, _D, _D], f16, kind="ExternalInput")
    d_wl = nc.dram_tensor("wdrl", [_NL, _D, _D], f8e5, kind="ExternalInput")
    d_wr = nc.dram_tensor("wdrh", [_NL, _D, _D], f8e5, kind="ExternalInput")
    d_bias = nc.dram_tensor("bias", [_P, (_NL - 1) * _KC], f32, kind="ExternalInput")
    d_out = nc.dram_tensor(
        "out", [2, _BT, _NBC, _P, _BCH], f16, kind="ExternalOutput"
    )

    with TileContext(nc) as tc:
        with (
            tc.tile_pool(name="const", bufs=1) as constp,
            tc.tile_pool(name="w", bufs=2) as wp,
            tc.tile_pool(name="v", bufs=2) as vp,
            tc.tile_pool(name="inp", bufs=1) as inp,
            tc.tile_pool(name="tmp", bufs=2) as tmp,
            tc.tile_pool(name="ps", bufs=1, space="PSUM") as ps,
        ):
            ident = constp.tile([_P, _P], f32)
            make_identity(nc, ident[:])
            bias_sb = constp.tile([_P, (_NL - 1) * _KC], f32)
            nc.sync.dma_start(bias_sb[:], d_bias[:])

            def fetch_wh(i):
                wh_t = wp.tile([_P, _KC, _D], f16, tag="wh")
                for h in range(4):
                    csl = slice(h * 2 * _P, (h + 1) * 2 * _P)
                    nc.sync.dma_start(
                        wh_t[:, 2 * h : 2 * h + 2, :],
                        d_wh[i, csl, :].rearrange("(c p) n -> p c n", p=_P),
                    )
                return wh_t

            def fetch_wlr(i):
                wl_t = wp.tile([_P, _KC, _D], f8e5, tag="wl")
                wr_t = wp.tile([_P, _KC, _D], f8e5, tag="wr")
                for h in range(2):
                    csl = slice(h * 4 * _P, (h + 1) * 4 * _P)
                    nc.sync.dma_start(
                        wl_t[:, 4 * h : 4 * h + 4, :],
                        d_wl[i, csl, :].rearrange("(c p) n -> p c n", p=_P),
                    )
                    nc.sync.dma_start(
                        wr_t[:, 4 * h : 4 * h + 4, :],
                        d_wr[i, csl, :].rearrange("(c p) n -> p c n", p=_P),
                    )
                return wl_t, wr_t

            def fetch_weights(i):
                wh_t = fetch_wh(i)
                wl_t, wr_t = fetch_wlr(i)
                return wh_t, wl_t, wr_t

            # ---- input: v0 = sign*exp(log_abs); fp16 split; DR-permuted vT ----
            xh = vp.tile([_P, _KC, _BP], f16, tag="xh")
            xl = vp.tile([_P, _KC, _BP], f16, tag="xl")
            pxl = vp.tile([_P, _KC, _BP], f8e5, tag="pxl")
            pxh = vp.tile([_P, _KC, _BP], f8e5, tag="pxh")
            for t in range(_BT):
                tsl = slice(t * _P, (t + 1) * _P)
                import contextlib
                depri = (
                    tc.high_priority(offset=-DEPRI)
                    if (t >= 4 and DEPRI)
                    else (tc.high_priority() if t == 0 else contextlib.nullcontext())
                )
                lab_t = inp.tile([_P, _D], f32, tag="lab", bufs=2)
                sgn_t = inp.tile([_P, _D], f8e5, tag="sgn", bufs=2)
                eng0 = nc.sync if t == 0 else nc.scalar
                with depri:
                    eng0.dma_start(lab_t[:], d_lab[tsl, :])
                    eng0.dma_start(sgn_t[:], d_sgn[tsl, :])
                if t == 1:
                    wh0_t = wp.tile([_P, _KC, _D], f16, tag="wh")
                    wl0_t = wp.tile([_P, _KC, _D], f8e5, tag="wl")
                    wr0_t = wp.tile([_P, _KC, _D], f8e5, tag="wr")
                    wtiles = (wh0_t, wl0_t, wr0_t)
                if t in (1, 3):
                    nh = slice(0, _BCH) if t == 1 else slice(_BCH, _D)
                    for h in range(2):
                        csl = slice(h * 4 * _P, (h + 1) * 4 * _P)
                        nc.sync.dma_start(
                            wh0_t[:, 4 * h : 4 * h + 4, nh],
                            d_wh[0, csl, nh].rearrange("(c p) n -> p c n", p=_P),
                        )
                if t in (2, 4):
                    nh = slice(0, _BCH) if t == 2 else slice(_BCH, _D)
                    nc.sync.dma_start(
                        wl0_t[:, :, nh],
                        d_wl[0, :, nh].rearrange("(c p) n -> p c n", p=_P),
                    )
                    nc.sync.dma_start(
                        wr0_t[:, :, nh],
                        d_wr[0, :, nh].rearrange("(c p) n -> p c n", p=_P),
                    )
                v_t = inp.tile([_P, _D], f32, tag="vt", bufs=2)
                if True:
                    nc.scalar.activation(v_t[:], lab_t[:], AF.Exp)
                    nc.vector.tensor_mul(out=v_t[:], in0=v_t[:], in1=sgn_t[:])
                    for blk in range(_KC):
                        st = 256 * (blk // 2) + (blk % 2)
                        dsl = slice(st, min(st + 256, _D), 2)
                        ptile = ps.tile([_P, _P], f32, tag="tr", bufs=3)
                        nc.tensor.transpose(ptile[:], v_t[:, dsl], ident[:])
                        nc.vector.tensor_copy(out=xh[:, blk, tsl], in_=ptile[:])
                        nc.vector.tensor_sub(
                            out=xl[:, blk, tsl], in0=ptile[:], in1=xh[:, blk, tsl]
                        )
                nc.scalar.activation(
                    pxl[:, :, tsl], xl[:, :, tsl], AF.Copy, scale=8.0
                )
                nc.scalar.activation(
                    pxh[:, :, tsl], xh[:, :, tsl], AF.Copy, scale=0.015625
                )

            # ---- 7 inner layers: v = tanh(v @ W + b) + v ----
            for i in range(_NL - 1):
                wh_t, wl_t, wr_t = wtiles
                if i + 1 < _NL - 1:
                    wtiles = fetch_weights(i + 1)
                xh_new = vp.tile([_P, _KC, _BP], f16, tag="xh")
                xl_new = vp.tile([_P, _KC, _BP], f16, tag="xl")
                pxl_new = vp.tile([_P, _KC, _BP], f8e5, tag="pxl")
                pxh_new = vp.tile([_P, _KC, _BP], f8e5, tag="pxh")
                for bc in range(_NBC):
                    bsl = slice(bc * _BCH, (bc + 1) * _BCH)
                    for n in range(_KC):
                        nsl = slice(n * _P, (n + 1) * _P)
                        s_t = tmp.tile([_P, _BCH], f32, tag="s", bufs=3)
                        nc.gpsimd.tensor_add(
                            out=s_t[:], in0=xh[:, n, bsl], in1=xl[:, n, bsl]
                        )
                        pt = ps.tile([_P, _BCH], f32, tag="mm", bufs=5)
                        for c in range(_KC):
                            nc.tensor.matmul(
                                pt[:], wh_t[:, c, nsl], xh[:, c, bsl],
                                start=(c == 0), stop=False,
                            )
                        for a in range(_KC // 2):
                            psl = slice(2 * a, 2 * a + 2)
                            nc.tensor.matmul(
                                pt[:], wl_t[:, psl, nsl], pxl[:, psl, bsl],
                                start=False, stop=False, perf_mode=DR,
                            )
                        for a in range(_KC // 2):
                            psl = slice(2 * a, 2 * a + 2)
                            nc.tensor.matmul(
                                pt[:], wr_t[:, psl, nsl], pxh[:, psl, bsl],
                                start=False, stop=(a == _KC // 2 - 1),
                                perf_mode=DR,
                            )
                        u = tmp.tile([_P, _BCH], f32, tag="u", bufs=3)
                        nc.scalar.activation(
                            u[:], pt[:], AF.Tanh,
                            bias=bias_sb[:, i * _KC + n : i * _KC + n + 1],
                        )
                        a_t = tmp.tile([_P, _BCH], f32, tag="a", bufs=3)
                        nc.vector.tensor_add(out=a_t[:], in0=u[:], in1=s_t[:])
                        nc.vector.tensor_copy(out=xh_new[:, n, bsl], in_=a_t[:])
                        nc.vector.tensor_sub(
                            out=xl_new[:, n, bsl], in0=a_t[:], in1=xh_new[:, n, bsl]
                        )
                        nc.scalar.activation(
                            pxl_new[:, n, bsl], xl_new[:, n, bsl], AF.Copy, scale=8.0
                        )
                        nc.gpsimd.tensor_scalar_mul(
                            out=pxh_new[:, n, bsl], in0=xh_new[:, n, bsl],
                            scalar1=0.015625,
                        )
                xh, xl, pxl, pxh = xh_new, xl_new, pxl_new, pxh_new

            # ---- final layer: t = v @ W_f, out = [sign(t), log|t|] ----
            whf = wp.tile([_P, _KC, _D], f16, tag="wh")
            nc.sync.dma_start(
                whf[:], d_wh[_NL - 1].rearrange("(c p) n -> p c n", p=_P)
            )
            wlf = wp.tile([_P, _KC, _D], f8e5, tag="wl")
            nc.sync.dma_start(
                wlf[:], d_wl[_NL - 1].rearrange("(c p) n -> p c n", p=_P)
            )
            wrf = wp.tile([_P, _KC, _D], f8e5, tag="wr")
            nc.sync.dma_start(
                wrf[:], d_wr[_NL - 1].rearrange("(c p) n -> p c n", p=_P)
            )
            for bt in range(_BT):
                bsl = slice(bt * _P, (bt + 1) * _P)
                for j in range(_NBC):
                    nsl = slice(j * _BCH, (j + 1) * _BCH)
                    pt = ps.tile([_P, _BCH], f32, tag="mm", bufs=5)
                    for c in range(_KC):
                        nc.tensor.matmul(
                            pt[:], xh[:, c, bsl], whf[:, c, nsl],
                            start=(c == 0), stop=False,
                        )
                    for a in range(_KC // 2):
                        psl = slice(2 * a, 2 * a + 2)
                        nc.tensor.matmul(
                            pt[:], pxl[:, psl, bsl], wlf[:, psl, nsl],
                            start=False, stop=False, perf_mode=DR,
                        )
                    for a in range(_KC // 2):
                        psl = slice(2 * a, 2 * a + 2)
                        nc.tensor.matmul(
                            pt[:], pxh[:, psl, bsl], wrf[:, psl, nsl],
                            start=False, stop=(a == _KC // 2 - 1), perf_mode=DR,
                        )
                    sg = tmp.tile([_P, _BCH], f16, tag="sgo", bufs=3)
                    ab = tmp.tile([_P, _BCH], f32, tag="a", bufs=3)
                    lg = tmp.tile([_P, _BCH], f16, tag="lgo", bufs=3)
                    nhalf = 1
                    for hh in range(nhalf):
                        hsl = slice(hh * _BCH // nhalf, (hh + 1) * _BCH // nhalf)
                        nc.scalar.activation(sg[:, hsl], pt[:, hsl], AF.Sign)
                        nc.vector.tensor_mul(
                            out=ab[:, hsl], in0=pt[:, hsl], in1=sg[:, hsl]
                        )
                        nc.scalar.activation(lg[:, hsl], ab[:, hsl], AF.Ln)
                        nc.sync.dma_start(d_out[0, bt, j, :, hsl], sg[:, hsl])
                        nc.sync.dma_start(d_out[1, bt, j, :, hsl], lg[:, hsl])
    nc.compile()
    return nc


def _perm():
    m = np.arange(_D)
    blk = m // _P
    q = m % _P
    j, i = blk // 2, blk % 2
    return 256 * j + 2 * q + i


def kernel(sign_x, log_abs_x, inner_kernels, final_kernel):
    global _cached_nc, last_results
    import ml_dtypes
    from concourse.bass_utils import run_bass_kernel_spmd

    E5 = ml_dtypes.float8_e5m2
    if _cached_nc is None:
        _cached_nc = _build()
    nc = _cached_nc

    sign_x = np.ascontiguousarray(np.asarray(sign_x, dtype=np.float32).astype(E5))
    log_abs_x = np.ascontiguousarray(np.asarray(log_abs_x, dtype=np.float32))
    ik = np.asarray(inner_kernels, dtype=np.float32)
    fk = np.asarray(final_kernel, dtype=np.float32)

    P = _perm()
    W = np.concatenate([ik[:, :_D, :], fk[None]], axis=0)  # [8, 1024, 1024]
    Wp = W[:, P, :]                       # rows to state order
    Wp[:_NL - 1] = Wp[:_NL - 1][:, :, P]  # inner cols to state order
    Wh = Wp.astype(np.float16)
    Wres = Wp - Wh.astype(np.float32)
    Wdrl = np.ascontiguousarray((Wh.astype(np.float32) * 0.125).astype(E5))
    Wdrh = np.ascontiguousarray((Wres * 64.0).astype(E5))
    Wh = np.ascontiguousarray(Wh)
    # bias pre-transposed to the SBUF layout [p, (layer, blk)], state order
    bias = ik[:, _D, :][:, P].reshape(_NL - 1, _KC, _P)
    bias = np.ascontiguousarray(bias.transpose(2, 0, 1).reshape(_P, (_NL - 1) * _KC))

    in_maps = []
    for cid in range(_NCORES):
        sl = slice(cid * _BP, (cid + 1) * _BP)
        in_maps.append({
            "sign_x": np.ascontiguousarray(sign_x[sl]),
            "log_abs_x": np.ascontiguousarray(log_abs_x[sl]),
            "wh": Wh,
            "wdrl": Wdrl,
            "wdrh": Wdrh,
            "bias": bias,
        })

    last_results = run_bass_kernel_spmd(nc, in_maps, core_ids=list(range(_NCORES)))
    outs = [
        r["out"].transpose(0, 1, 3, 2, 4).reshape(2, _BP, _D).astype(np.float32)
        for r in last_results.results
    ]
    return np.concatenate(outs, axis=1)
